# revision 17
# baseline (speedup 1.0000x reference)
"""Trainium2 Bass kernel for nn_AttentionModel (GRU encoder + attention decoder).

Mathematical reduction: the model output is outs[i] = logp[0] of decoder step i,
and every decoder quantity for batch row b depends only on batch row b (the GRU
cell, attention, argmax feedback are all row-wise).  enc_hidden feeds the
decoder only through row 0, and enc_vecs comes from batch row 0 of the encoder.
So the exact full-model output equals a batch-1 computation: a 2048-step GRU
over batch row 0's token stream, then a 512-step greedy decoder on row 0.

On-device: everything except the final log-softmax normalization (the argmax
feedback uses raw logits, which is equivalent; the -logsumexp shift is applied
on the host in float64, well inside fp32 tolerance).
"""

import os
import sys
from contextlib import ExitStack

import numpy as np

sys.path.insert(0, "/opt/trn_rl_repo")

H = 128
MAX_LEN = 512
INTER = 16
F = 128
B = 512
OBS_VOCAB = 2048
A = 512

ENC_STEPS = INTER * F  # 2048
DEC_STEPS = B  # 512

_cache = {}


def _build(enc_steps, dec_steps):
    import concourse.bass as bass
    import concourse.bacc as bacc
    import concourse.mybir as mybir
    import concourse.tile as tile
    from concourse.tile_rust import add_dep_helper

    dt = mybir.dt
    f32 = dt.float32
    f32r = dt.float32r
    bf16 = dt.bfloat16
    u32 = dt.uint32
    i32 = dt.int32
    AF = mybir.ActivationFunctionType
    OP = mybir.AluOpType
    n_chunks = enc_steps // F

    nc = bacc.Bacc("TRN2", target_bir_lowering=False, debug=False)

    def din(name, shape, dtype=f32):
        return nc.dram_tensor(name, shape, dtype, kind="ExternalInput").ap()

    tokens_T = din("tokens_T", (F, n_chunks), i32)
    enc_embed = din("enc_embed", (OBS_VOCAB, H))
    identity = din("identity", (H, H))
    Whh_r = din("Whh_r", (H, H))
    Whh_zn = din("Whh_zn", (H, H))
    Whh_n = din("Whh_n", (H, H))
    Wih_r = din("Wih_r", (H, H))
    Wih_zn = din("Wih_zn", (H, H))
    Wih_n = din("Wih_n", (H, H))
    hbr = din("hbr", (H, 1))
    hbz = din("hbz", (H, 1))
    bn_p = din("bn_p", (H, 1))
    hbhn = din("hbhn", (H, 1))
    dWih_r = din("dWih_r", (H, H))
    dWih_zn = din("dWih_zn", (H, H))
    dWih_n = din("dWih_n", (H, H))
    dWhh_r = din("dWhh_r", (H, H))
    dWhh_zn = din("dWhh_zn", (H, H))
    dWhh_n = din("dWhh_n", (H, H))
    dec_brz_half = din("dec_brz_half", (H, 2))
    dbihn = din("dbihn", (H, 1))
    dhbhn = din("dhbhn", (H, 1))
    attn_top = din("attn_top", (H, MAX_LEN), bf16)
    attn_bot = din("attn_bot", (H, MAX_LEN), bf16)
    attnb_mat = din("attnb_mat", (H, MAX_LEN), bf16)
    e1vec = din("e1vec", (H, 1), bf16)
    attn16_top = din("attn16_top", (H, INTER))
    attn16_bot = din("attn16_bot", (H, INTER))
    ab16 = din("ab16", (INTER, 1))
    comb_top = din("comb_top", (H, H))
    comb_bot = din("comb_bot", (H, H))
    comb_b = din("comb_b", (H, 1))
    outW = [din(f"outW{j}", (H, H)) for j in range(4)]
    outb_cols = din("outb_cols", (H, 4))
    dec_embT = din("dec_embT", (H, A))
    iota_p = din("iota_p", (H, 1))
    ones_row = din("ones_row", (1, H))

    out_L = nc.dram_tensor("out", (A, dec_steps), f32, kind="ExternalOutput").ap()

    with ExitStack() as ctx:
        tc = ctx.enter_context(tile.TileContext(nc))
        # ---- persistent SBUF pools
        wpool = ctx.enter_context(tc.tile_pool(name="weights", bufs=1))
        gipool = ctx.enter_context(tc.tile_pool(name="gi", bufs=1))
        state = ctx.enter_context(tc.tile_pool(name="state", bufs=3))
        scratch = ctx.enter_context(tc.tile_pool(name="scratch", bufs=2))

        def load(ap_dram, shape, dtype=f32):
            t = wpool.tile(list(shape), dtype, tag=f"w_{ap_dram.tensor.name}")
            if dtype != ap_dram.dtype:
                nc.sync.dma_start(t[:].bitcast(ap_dram.dtype), ap_dram[:])
            else:
                nc.sync.dma_start(t[:], ap_dram[:])
            return t

        tokT_sb = load(tokens_T, (F, n_chunks), i32)
        ident_sb = load(identity, (H, H))
        sWhh_r = load(Whh_r, (H, H))
        sWhh_zn = load(Whh_zn, (H, H))
        sWhh_n = load(Whh_n, (H, H))
        sWih_r = load(Wih_r, (H, H))
        sWih_zn = load(Wih_zn, (H, H))
        sWih_n = load(Wih_n, (H, H))
        s_hbr = load(hbr, (H, 1))
        s_hbz = load(hbz, (H, 1))
        s_bn_p = load(bn_p, (H, 1))
        s_hbhn = load(hbhn, (H, 1))
        sdWih_r = load(dWih_r, (H, H))
        sdWih_zn = load(dWih_zn, (H, H))
        sdWih_n = load(dWih_n, (H, H))
        sdWhh_r = load(dWhh_r, (H, H))
        sdWhh_zn = load(dWhh_zn, (H, H))
        sdWhh_n = load(dWhh_n, (H, H))
        s_dbrz = load(dec_brz_half, (H, 2))
        s_dbihn = load(dbihn, (H, 1))
        s_dhbhn = load(dhbhn, (H, 1))
        s_attop = load(attn_top, (H, MAX_LEN), bf16)
        s_atbot = load(attn_bot, (H, MAX_LEN), bf16)
        s_atbmat = load(attnb_mat, (H, MAX_LEN), bf16)
        s_e1 = load(e1vec, (H, 1), bf16)
        s_a16t = load(attn16_top, (H, INTER))
        s_a16b = load(attn16_bot, (H, INTER))
        s_ab16 = load(ab16, (INTER, 1))
        s_combt = load(comb_top, (H, H))
        s_combb = load(comb_bot, (H, H))
        s_comb_b = load(comb_b, (H, 1))
        s_outW = [load(outW[j], (H, H)) for j in range(4)]
        s_outb = load(outb_cols, (H, 4))
        s_dembT = load(dec_embT, (H, A))
        s_iota = load(iota_p, (H, 1))
        s_ones = load(ones_row, (1, H))

        def hilo(t, shape, name):
            hi = wpool.tile(list(shape), bf16, tag=f"hi_{name}")
            nc.vector.tensor_copy(hi[:], t[:])
            lo = wpool.tile(list(shape), bf16, tag=f"lo_{name}")
            nc.vector.tensor_tensor(lo[:], t[:], hi[:], op=OP.subtract)
            return hi, lo

        Whh_hl = {
            c: hilo(w, (H, H), f"Whh{c}")
            for c, w in (("r", sWhh_r), ("z", sWhh_zn), ("n", sWhh_n))
        }
        dWih_hl = {
            c: hilo(w, (H, H), f"dWih{c}")
            for c, w in (("r", sdWih_r), ("z", sdWih_zn), ("n", sdWih_n))
        }
        dWhh_hl = {
            c: hilo(w, (H, H), f"dWhh{c}")
            for c, w in (("r", sdWhh_r), ("z", sdWhh_zn), ("n", sdWhh_n))
        }
        combt_hl = hilo(s_combt, (H, H), "combt")
        combb_hl = hilo(s_combb, (H, H), "combb")
        outW_hl = [hilo(s_outW[j], (H, H), f"outW{j}") for j in range(4)]
        ones_bf = wpool.tile([1, H], bf16, tag="ones_bf")
        nc.vector.tensor_copy(ones_bf[:], s_ones[:])

        def mm3(psum_ap, w_hl, v_hi, v_lo, first=True, last=True):
            whi, wlo = w_hl
            nc.tensor.matmul(psum_ap, whi[:], v_hi[:], start=first, stop=False)
            nc.tensor.matmul(psum_ap, whi[:], v_lo[:], start=False, stop=False)
            nc.tensor.matmul(psum_ap, wlo[:], v_hi[:], start=False, stop=last)

        gi_rz = gipool.tile([H, 2 * enc_steps], f32)
        gi_n = gipool.tile([H, enc_steps], f32)
        xT = gipool.tile([H, enc_steps], f32)
        encv = gipool.tile([H, INTER], f32)
        v16 = gipool.tile([INTER, H], f32)
        buf = gipool.tile([H, 4 * dec_steps], f32)
        lb8 = gipool.tile([H, 8], f32)
        nc.vector.memset(lb8[:, 4:8], -1e30)
        nc.vector.memset(encv[:], 0.0)

        # ================= embedding gather + gi precompute =================
        with tc.tile_pool(name="pre_ps", bufs=2, space="PSUM") as pps, tc.tile_pool(
            name="pre_sb", bufs=3
        ) as psb:
            for t in range(n_chunks):
                Xg = psb.tile([F, H], f32, tag="Xg")
                nc.gpsimd.indirect_dma_start(
                    out=Xg[:],
                    out_offset=None,
                    in_=enc_embed[:],
                    in_offset=bass.IndirectOffsetOnAxis(
                        ap=tokT_sb[:, t : t + 1], axis=0
                    ),
                )
                pxt = pps.tile([H, F], f32, tag="pxt")
                nc.tensor.transpose(pxt[:], Xg[:], ident_sb[:])
                nc.scalar.activation(
                    xT[:, t * F : (t + 1) * F], pxt[:], AF.Identity
                )
            gi_rz_v = gi_rz[:].rearrange("p (k g) -> p g k", g=2)
            for (W, scale, bias, dst) in (
                (sWih_r, 0.5, s_hbr, 0),
                (sWih_zn, 0.5, s_hbz, 1),
                (sWih_n, 1.0, s_bn_p, 2),
            ):
                for t in range(n_chunks):
                    pgi = pps.tile([H, F], f32, tag="pgi")
                    nc.tensor.matmul(
                        pgi[:],
                        W[:],
                        xT[:, t * F : (t + 1) * F],
                        start=True,
                        stop=True,
                    )
                    if dst == 2:
                        o_ap = gi_n[:, t * F : (t + 1) * F]
                    else:
                        o_ap = gi_rz_v[:, dst, t * F : (t + 1) * F]
                    nc.scalar.activation(
                        o_ap, pgi[:], AF.Identity, bias=bias[:], scale=scale
                    )

        # ================= encoder recurrence =================
        h_cur = state.tile([H, 1], f32, tag="h")
        nc.vector.memset(h_cur[:], 0.0)
        h_hi = state.tile([H, 1], bf16, tag="hh")
        nc.vector.memset(h_hi[:], 0.0)
        h_lo = state.tile([H, 1], bf16, tag="hl")
        nc.vector.memset(h_lo[:], 0.0)

        def gru_h_split(h_new):
            nh = state.tile([H, 1], bf16, tag="hh")
            nc.vector.tensor_copy(nh[:], h_new[:])
            nl = state.tile([H, 1], bf16, tag="hl")
            nc.vector.tensor_tensor(nl[:], h_new[:], nh[:], op=OP.subtract)
            return nh, nl

        with tc.tile_pool(name="enc_ps", bufs=2, space="PSUM") as eps:
            for k in range(enc_steps):
                pg = eps.tile([H, 4], f32, tag="pg")
                for c, col in (("n", 2), ("r", 0), ("z", 1)):
                    mm3(pg[:, col : col + 1], Whh_hl[c], h_hi, h_lo)
                va = scratch.tile([H, 2], f32, tag="va")
                nc.vector.scalar_tensor_tensor(
                    va[:], pg[:, 0:2], 0.5, gi_rz[:, 2 * k : 2 * k + 2],
                    OP.mult, OP.add,
                )
                t3 = scratch.tile([H, 1], f32, tag="t3")
                nc.vector.scalar_tensor_tensor(
                    t3[:], pg[:, 2:3], 0.5, s_hbhn[:], OP.mult, OP.add
                )
                t4 = scratch.tile([H, 1], f32, tag="t4")
                nc.vector.scalar_tensor_tensor(
                    t4[:], pg[:, 2:3], 0.5, gi_n[:, k : k + 1], OP.mult, OP.add
                )
                w2 = scratch.tile([H, 2], f32, tag="w2")
                nc.scalar.activation(w2[:], va[:], AF.Tanh)
                nt = scratch.tile([H, 1], f32, tag="nt")
                nc.scalar.activation(
                    nt[:], t3[:], AF.Tanh, bias=t4[:], scale=w2[:, 0:1]
                )
                d = scratch.tile([H, 1], f32, tag="d")
                nc.vector.tensor_tensor(d[:], nt[:], h_cur[:], op=OP.subtract)
                s1 = scratch.tile([H, 1], f32, tag="s1")
                nc.vector.scalar_tensor_tensor(
                    s1[:], d[:], w2[:, 1:2], d[:], OP.mult, OP.add
                )
                h_hi = state.tile([H, 1], bf16, tag="hh")
                nc.vector.scalar_tensor_tensor(
                    h_hi[:], s1[:], 0.5, h_cur[:], OP.mult, OP.add
                )
                h_new = state.tile([H, 1], f32, tag="h")
                nc.vector.scalar_tensor_tensor(
                    h_new[:], s1[:], 0.5, h_cur[:], OP.mult, OP.add
                )
                h_lo = state.tile([H, 1], bf16, tag="hl")
                nc.vector.tensor_tensor(h_lo[:], h_new[:], h_hi[:], op=OP.subtract)
                if k % F == 0:
                    nc.vector.tensor_copy(encv[:, k // F : k // F + 1], h_new[:])
                h_cur = h_new

        # ================= decoder =================
        with tc.tile_pool(name="dec_ps", bufs=1, space="PSUM") as dps, tc.tile_pool(
            name="dec_ps2", bufs=1, space="PSUM"
        ) as dps2:
            pv16 = dps.tile([INTER, H], f32, tag="pv16")
            nc.tensor.transpose(pv16[:], encv[:], ident_sb[:])
            nc.scalar.activation(v16[:], pv16[:], AF.Identity)
            v16_hl = hilo(v16, (INTER, H), "v16")

            e_cur = state.tile([H, 1], f32, tag="e")
            nc.vector.tensor_copy(e_cur[:], s_dembT[:, 0:1])
            e_hi = state.tile([H, 1], bf16, tag="eh")
            nc.vector.tensor_copy(e_hi[:], e_cur[:])
            e_lo = state.tile([H, 1], bf16, tag="el")
            nc.vector.tensor_tensor(e_lo[:], e_cur[:], e_hi[:], op=OP.subtract)

            buf_v = buf[:].rearrange("p (j k) -> p k j", j=4)

            for k in range(dec_steps):
                # ---- attention denominator (full 512 logits as a row)
                ps_row = dps.tile([1, MAX_LEN], f32, tag="srow")
                nc.tensor.matmul(ps_row[:], s_e1[:], s_atbmat[:], start=True, stop=False)
                nc.tensor.matmul(ps_row[:], e_hi[:], s_attop[:], start=False, stop=False)
                nc.tensor.matmul(ps_row[:], h_hi[:], s_atbot[:], start=False, stop=True)
                # ---- first-16 attention logits (column form)
                pA = dps.tile([H, 2], f32, tag="pA")
                nc.tensor.matmul(
                    pA[0:INTER, 0:1], s_a16t[:], e_cur[:], start=True, stop=False
                )
                nc.tensor.matmul(
                    pA[0:INTER, 0:1], s_a16b[:], h_cur[:], start=False, stop=True
                )
                p16 = scratch.tile([INTER, 1], f32, tag="p16")
                nc.scalar.activation(
                    p16[:], pA[0:INTER, 0:1], AF.Exp, bias=s_ab16[:]
                )
                exps = scratch.tile([1, MAX_LEN], f32, tag="exps")
                S_sb = scratch.tile([1, 1], f32, tag="S")
                nc.scalar.activation(exps[:], ps_row[:], AF.Exp, accum_out=S_sb[:])
                rs = scratch.tile([1, 1], f32, tag="rs")
                nc.vector.reciprocal(rs[:], S_sb[:])
                rs_hi = scratch.tile([1, 1], bf16, tag="rs_hi")
                nc.vector.tensor_copy(rs_hi[:], rs[:])
                rs_lo = scratch.tile([1, 1], bf16, tag="rs_lo")
                nc.vector.tensor_tensor(rs_lo[:], rs[:], rs_hi[:], op=OP.subtract)
                pR = dps.tile([H, 1], f32, tag="pR")
                nc.tensor.matmul(pR[:], ones_bf[:], rs_hi[:], start=True, stop=False)
                nc.tensor.matmul(pR[:], ones_bf[:], rs_lo[:], start=False, stop=True)
                rsb = scratch.tile([H, 1], f32, tag="rsb")
                nc.vector.tensor_copy(rsb[:], pR[:])
                # ---- applied = enc_vecs^T @ p16 (unnormalized)
                p16h = scratch.tile([INTER, 1], bf16, tag="p16h")
                nc.vector.tensor_copy(p16h[:], p16[:])
                p16l = scratch.tile([INTER, 1], bf16, tag="p16l")
                nc.vector.tensor_tensor(p16l[:], p16[:], p16h[:], op=OP.subtract)
                mm3(pA[:, 1:2], v16_hl, p16h, p16l)
                ap_hi = scratch.tile([H, 1], bf16, tag="ap_hi")
                nc.vector.tensor_copy(ap_hi[:], pA[:, 1:2])
                ap_lo = scratch.tile([H, 1], bf16, tag="ap_lo")
                nc.vector.tensor_tensor(ap_lo[:], pA[:, 1:2], ap_hi[:], op=OP.subtract)
                # ---- comb + relu
                pU = dps.tile([H, 2], f32, tag="pU")
                mm3(pU[:, 0:1], combt_hl, e_hi, e_lo)
                mm3(pU[:, 1:2], combb_hl, ap_hi, ap_lo)
                b2 = scratch.tile([H, 1], f32, tag="b2")
                nc.vector.tensor_scalar(
                    b2[:], pU[:, 0:1], s_comb_b[:], None, OP.add
                )
                o = scratch.tile([H, 1], f32, tag="o")
                nc.scalar.activation(
                    o[:], pU[:, 1:2], AF.Relu, bias=b2[:], scale=rsb[:]
                )
                o_hi = scratch.tile([H, 1], bf16, tag="o_hi")
                nc.vector.tensor_copy(o_hi[:], o[:])
                o_lo = scratch.tile([H, 1], bf16, tag="o_lo")
                nc.vector.tensor_tensor(o_lo[:], o[:], o_hi[:], op=OP.subtract)
                # ---- GRU cell
                pG = dps2.tile([H, 4], f32, tag="pG")
                mm3(pG[:, 2:3], dWhh_hl["n"], h_hi, h_lo)
                mm3(pG[:, 3:4], dWih_hl["n"], o_hi, o_lo)
                mm3(pG[:, 0:1], dWih_hl["r"], o_hi, o_lo, last=False)
                mm3(pG[:, 0:1], dWhh_hl["r"], h_hi, h_lo, first=False)
                mm3(pG[:, 1:2], dWih_hl["z"], o_hi, o_lo, last=False)
                mm3(pG[:, 1:2], dWhh_hl["z"], h_hi, h_lo, first=False)
                t3 = scratch.tile([H, 1], f32, tag="t3")
                nc.vector.scalar_tensor_tensor(
                    t3[:], pG[:, 2:3], 0.5, s_dhbhn[:], OP.mult, OP.add
                )
                t4 = scratch.tile([H, 1], f32, tag="t4")
                nc.vector.scalar_tensor_tensor(
                    t4[:], pG[:, 3:4], s_dbihn[:], t3[:], OP.add, OP.add
                )
                va = scratch.tile([H, 2], f32, tag="va")
                nc.vector.scalar_tensor_tensor(
                    va[:], pG[:, 0:2], 0.5, s_dbrz[:], OP.mult, OP.add
                )
                w2 = scratch.tile([H, 2], f32, tag="w2")
                nc.scalar.activation(w2[:], va[:], AF.Tanh)
                nt = scratch.tile([H, 1], f32, tag="nt")
                nc.scalar.activation(
                    nt[:], t3[:], AF.Tanh, bias=t4[:], scale=w2[:, 0:1]
                )
                d = scratch.tile([H, 1], f32, tag="d")
                nc.vector.tensor_tensor(d[:], nt[:], h_cur[:], op=OP.subtract)
                s1 = scratch.tile([H, 1], f32, tag="s1")
                nc.vector.scalar_tensor_tensor(
                    s1[:], d[:], w2[:, 1:2], d[:], OP.mult, OP.add
                )
                nh_hi = state.tile([H, 1], bf16, tag="hh")
                nc.vector.scalar_tensor_tensor(
                    nh_hi[:], s1[:], 0.5, h_cur[:], OP.mult, OP.add
                )
                h_new = state.tile([H, 1], f32, tag="h")
                nc.vector.scalar_tensor_tensor(
                    h_new[:], s1[:], 0.5, h_cur[:], OP.mult, OP.add
                )
                nh_lo = state.tile([H, 1], bf16, tag="hl")
                nc.vector.tensor_tensor(nh_lo[:], h_new[:], nh_hi[:], op=OP.subtract)
                # ---- output logits (column-major, 4 blocks of 128)
                pL = dps2.tile([H, 4], f32, tag="pL")
                for j in range(4):
                    mm3(pL[:, j : j + 1], outW_hl[j], nh_hi, nh_lo)
                nc.vector.tensor_tensor(
                    lb8[:, 0:4], pL[:, 0:4], s_outb[:], op=OP.add
                )
                nc.vector.tensor_copy(buf_v[:, k, :], lb8[:, 0:4])
                # ---- argmax over the 512 logits -> e_next
                m8 = scratch.tile([H, 8], f32, tag="m8")
                nc.vector.max(m8[:], lb8[:])
                ji = scratch.tile([H, 8], u32, tag="ji")
                nc.vector.max_index(ji[:], m8[:], lb8[:])
                vf = scratch.tile([H, 1], f32, tag="vf")
                nc.vector.scalar_tensor_tensor(
                    vf[:], ji[:, 0:1], 128.0, s_iota[:], OP.mult, OP.add
                )
                pT = dps.tile([1, 2 * H], f32, tag="pT")
                nc.tensor.transpose(pT[:, 0:H], m8[:, 0:1], ident_sb[:])
                nc.tensor.transpose(pT[:, H : 2 * H], vf[:], ident_sb[:])
                g8 = scratch.tile([1, 8], f32, tag="g8")
                nc.vector.max(g8[:], pT[0:1, 0:H])
                gi8 = scratch.tile([1, 8], u32, tag="gi8")
                nc.vector.max_index(gi8[:], g8[:], pT[0:1, 0:H])
                e_new = state.tile([H, 1], f32, tag="e")
                cu = scratch.tile([1, 1], u32, tag="cu")
                reg_p = nc.alloc_register(mybir.EngineType.DVE, f"rp{k}")
                i1 = nc.vector.reg_load(reg_p, gi8[0:1, 0:1])
                i2 = nc.vector.reg_alu(reg_p, reg_p, 127, OP.bitwise_and)
                add_dep_helper(i2.ins, i1.ins, sync=False, reason="regp order")
                p_sv = nc.snap(reg_p, donate=True, min_val=0, max_val=127)
                i3 = nc.vector.tensor_copy(
                    cu[:], pT[0:1, H : 2 * H][:, bass.DynSlice(p_sv, 1)]
                )
                add_dep_helper(i3.ins, i2.ins, sync=False, reason="cu after mask")
                reg_v = nc.alloc_register(mybir.EngineType.DVE, f"rv{k}")
                i4 = nc.vector.reg_load(reg_v, cu[0:1, 0:1])
                i5 = nc.vector.reg_alu(reg_v, reg_v, 511, OP.bitwise_and)
                add_dep_helper(i5.ins, i4.ins, sync=False, reason="regv order")
                v_sv = nc.snap(reg_v, donate=True, min_val=0, max_val=511)
                i6 = nc.vector.tensor_copy(
                    e_new[:], s_dembT[:, bass.DynSlice(v_sv, 1)]
                )
                add_dep_helper(i6.ins, i5.ins, sync=False, reason="e after mask")
                e_hi = state.tile([H, 1], bf16, tag="eh")
                nc.vector.tensor_copy(e_hi[:], e_new[:])
                e_lo = state.tile([H, 1], bf16, tag="el")
                nc.vector.tensor_tensor(e_lo[:], e_new[:], e_hi[:], op=OP.subtract)
                h_cur = h_new
                h_hi, h_lo = nh_hi, nh_lo
                e_cur = e_new

        # ---- write out
        for j in range(4):
            nc.sync.dma_start(
                out_L[j * H : (j + 1) * H, :],
                buf[:, j * dec_steps : (j + 1) * dec_steps],
            )

    nc.compile()
    return nc


def _prep(inputs, enc_steps=ENC_STEPS, dec_steps=DEC_STEPS):
    import ml_dtypes

    bf = ml_dtypes.bfloat16
    f = np.float32
    obs = np.asarray(inputs["obs"])
    n_chunks = enc_steps // F
    toks = np.stack([obs[c * 32, :F] for c in range(n_chunks)], 0)  # (chunks, F)
    enc_Wih = np.asarray(inputs["enc_Wih"], f)
    enc_Whh = np.asarray(inputs["enc_Whh"], f)
    enc_bih = np.asarray(inputs["enc_bih"], f)
    enc_bhh = np.asarray(inputs["enc_bhh"], f)
    dec_Wih = np.asarray(inputs["dec_Wih"], f)
    dec_Whh = np.asarray(inputs["dec_Whh"], f)
    dec_bih = np.asarray(inputs["dec_bih"], f)
    dec_bhh = np.asarray(inputs["dec_bhh"], f)
    attn_W = np.asarray(inputs["attn_W"], f)
    attn_b = np.asarray(inputs["attn_b"], f)
    comb_W = np.asarray(inputs["comb_W"], f)
    comb_b = np.asarray(inputs["comb_b"], f)
    out_W = np.asarray(inputs["out_W"], f)
    out_b = np.asarray(inputs["out_b"], f)
    dec_embed = np.asarray(inputs["dec_embed"], f)

    c = lambda a: np.ascontiguousarray(a, f)
    attnb_mat = np.zeros((H, MAX_LEN), bf)
    attnb_mat[0, :] = attn_b.astype(bf)
    e1vec = np.zeros((H, 1), bf)
    e1vec[0, 0] = 1.0
    outb_cols = out_b.reshape(4, H).T
    dec_brz_half = np.stack(
        [
            0.5 * (dec_bih[0:H] + dec_bhh[0:H]),
            -0.5 * (dec_bih[H : 2 * H] + dec_bhh[H : 2 * H]),
        ],
        1,
    )
    dev = {
        "tokens_T": np.ascontiguousarray(toks.T, np.int32),
        "enc_embed": c(np.asarray(inputs["enc_embed"], f)),
        "identity": np.eye(H, dtype=f),
        "Whh_r": c(enc_Whh[:, 0:H]),
        "Whh_zn": c(-enc_Whh[:, H : 2 * H]),
        "Whh_n": c(enc_Whh[:, 2 * H : 3 * H]),
        "Wih_r": c(enc_Wih[:, 0:H]),
        "Wih_zn": c(-enc_Wih[:, H : 2 * H]),
        "Wih_n": c(enc_Wih[:, 2 * H : 3 * H]),
        "hbr": c(0.5 * (enc_bih[0:H] + enc_bhh[0:H])).reshape(H, 1),
        "hbz": c(-0.5 * (enc_bih[H : 2 * H] + enc_bhh[H : 2 * H])).reshape(H, 1),
        "bn_p": c(enc_bih[2 * H :] + 0.5 * enc_bhh[2 * H :]).reshape(H, 1),
        "hbhn": c(0.5 * enc_bhh[2 * H :]).reshape(H, 1),
        "dWih_r": c(dec_Wih[:, 0:H]),
        "dWih_zn": c(-dec_Wih[:, H : 2 * H]),
        "dWih_n": c(dec_Wih[:, 2 * H : 3 * H]),
        "dWhh_r": c(dec_Whh[:, 0:H]),
        "dWhh_zn": c(-dec_Whh[:, H : 2 * H]),
        "dWhh_n": c(dec_Whh[:, 2 * H : 3 * H]),
        "dec_brz_half": c(dec_brz_half),
        "dbihn": c(dec_bih[2 * H :]).reshape(H, 1),
        "dhbhn": c(0.5 * dec_bhh[2 * H :]).reshape(H, 1),
        "attn_top": np.ascontiguousarray(attn_W[0:H, :], bf),
        "attn_bot": np.ascontiguousarray(attn_W[H:, :], bf),
        "attnb_mat": attnb_mat,
        "e1vec": e1vec,
        "attn16_top": c(attn_W[0:H, 0:INTER]),
        "attn16_bot": c(attn_W[H:, 0:INTER]),
        "ab16": c(attn_b[0:INTER]).reshape(INTER, 1),
        "comb_top": c(comb_W[0:H, :]),
        "comb_bot": c(comb_W[H:, :]),
        "comb_b": c(comb_b).reshape(H, 1),
        "outb_cols": c(outb_cols),
        "dec_embT": c(dec_embed.T),
        "iota_p": np.arange(H, dtype=f).reshape(H, 1),
        "ones_row": np.ones((1, H), f),
    }
    for j in range(4):
        dev[f"outW{j}"] = c(out_W[:, j * H : (j + 1) * H])
    return dev


def _postprocess(L):
    # L is (512 vocab, steps); output logp = (steps, vocab) with log_softmax
    x = L.T.astype(np.float64)
    m = x.max(axis=1, keepdims=True)
    lse = np.log(np.exp(x - m).sum(axis=1, keepdims=True)) + m
    return (x - lse).astype(np.float32)


def _enable_ldw_opt():
    import concourse.bass_utils as bu

    return  # walrus codegen crashes with ldw-opt=true; keep default
    if getattr(bu, "_ldw_opt_patched", False):
        return
    orig = bu.bir_verify_and_optimise

    def patched(*a, **k):
        orig_run = bu.run_command

        def run2(cmd, **kw):
            cmd = [
                c.replace("--enable-ldw-opt=false", "--enable-ldw-opt=true")
                if isinstance(c, str)
                else c
                for c in cmd
            ]
            return orig_run(cmd, **kw)

        bu.run_command = run2
        try:
            return orig(*a, **k)
        finally:
            bu.run_command = orig_run

    bu.bir_verify_and_optimise = patched
    bu._ldw_opt_patched = True


def run_on_hw(inputs, enc_steps=ENC_STEPS, dec_steps=DEC_STEPS, trace=False):
    import concourse.bass_utils as bass_utils

    _enable_ldw_opt()

    key = (enc_steps, dec_steps)
    if key not in _cache:
        _cache[key] = _build(enc_steps, dec_steps)
    nc = _cache[key]
    dev = _prep(inputs, enc_steps, dec_steps)
    res = bass_utils.run_bass_kernel_spmd(
        nc, [dev] * 8, core_ids=list(range(8)), trace=trace
    )
    L = res.results[0]["out"]
    return _postprocess(L), res


def kernel(**inputs) -> np.ndarray:
    out, _ = run_on_hw(inputs)
    return out
